# revision 76
# baseline (speedup 1.0000x reference)
"""Trainium2 Bass kernel for nn_Decision_Node (Linear+Hardtanh -> sp, 2-class
softmax Gini -> gini), data-parallel over 8 NeuronCores.

Math per core shard (B_s=128 of B=1024 batches, T=128, F=784, L=256, C=2):
    sp   = clip(x @ W.T + b, -1, 1)                      [N=16384, 256]
    gini = 1.5 - 0.5*tanh(sp*d/2)^2,  d = contrib[...,0]-contrib[...,1]

Device strategy (v2, W-stationary):
  - x cast to fp16 on host, column-blocked+padded to [7, 128, N] with a
    bias-fold row (xt[6,16,:] = 1.0 pairs with wt[6,16,:] = b).
  - Matmuls keep W tiles stationary ([128f x 128l] from resident SBUF) and
    stream 512-row x chunks; out psum is [128l x 512r] (one full bank).
  - ACT reads each psum chunk once: sp_u8 = sat_u8(round(127.5*y + 127.5)).
    The u8 SATURATION implements the hardtanh clip; round-to-nearest was
    verified on HW. Host dequant: sp = (u8-127.5)/127.5 (max err 3.9e-3).
  - DVE scalar_tensor_tensor: z = (sp_u8 - 127.5) * (d/127.5)  [= sp*d].
  - ACT: th = tanh(z/2).  DVE stt: gini_u8 = (th*255)*th (round+sat).
    Host: gini = 1.5 - 0.5*u8/255.
  - Outputs staged [2(l-half), 128(l), N(rows)] u8; 4 KiB-per-partition
    stores every 4096 rows. Host transposes u8 slabs on dequant.
  - Blocks 1024/2048 rows: small first blocks cut the DMA ramp, small last
    blocks shorten the drain tail.
"""

import os
import sys
import types
from concurrent.futures import ThreadPoolExecutor

import numpy as np

for _p in (
    "/opt/trn_rl_repo",
    "/root/.axon_site",
    "/root/.axon_site/_ro/trn_rl_repo",
    "/root/.axon_site/_ro/pypackages",
):
    if os.path.isdir(_p) and _p not in sys.path:
        sys.path.append(_p)

B, T, F, L = 1024, 128, 784, 256
NCORES = 8
BS = B // NCORES          # batches per core
NROWS = BS * T            # 16384 rows per core
KT = 7                    # contraction tiles (784 = 6*128 + 16, padded)
KP = 17                   # used partitions in the last (remainder+bias) k-tile
CG = 2048                 # compute-group rows (elementwise op width)
CH = 512                  # matmul chunk rows (one psum bank)
DZW = 1024                # dz tile reps (d pattern is 128-periodic in rows)
BLOCKS = [512, 1024, 1536] + [2048] * 6 + [1024]

# flat blocked x layout: per block, k-tiles 0..5 are [128, nb] and the
# remainder k-tile is [KP, nb], all concatenated contiguously so every DMA
# is a fully linear HBM read.
XT_OFF = {}
_o = 0
for _bi, _nb in enumerate(BLOCKS):
    for _k in range(KT - 1):
        XT_OFF[(_bi, _k)] = _o
        _o += 128 * _nb
    XT_OFF[(_bi, KT - 1)] = _o
    _o += KP * _nb
XT_TOT = _o

# compute groups (small final groups shorten the post-matmul drain)
CGS = [CG] * 7 + [1024, 512, 512]
CG_OFF = []
_o = 0
for _w in CGS:
    CG_OFF.append(_o)
    _o += _w
assert _o == NROWS

# flat blocked output layout: per cg, per l-half, a contiguous [128, w] chunk
OUT_OFF = {}
_o = 0
for _m, _w in enumerate(CGS):
    for _nh in range(2):
        OUT_OFF[(_m, _nh)] = _o
        _o += 128 * _w
OUT_TOT = _o
assert OUT_TOT == 2 * 128 * NROWS


def _build_module():
    """Build + compile the single-core Bass/Tile module (SPMD across cores)."""
    import concourse.tile as tile
    from concourse import bacc, mybir

    f32, f16, u8 = mybir.dt.float32, mybir.dt.float16, mybir.dt.uint8
    Alu = mybir.AluOpType
    Act = mybir.ActivationFunctionType

    nc = bacc.Bacc(
        "TRN2",
        target_bir_lowering=False,
        debug=False,
        enable_asserts=False,
        num_devices=NCORES,
    )
    # All dram tensors are laid out by the host so every DMA is a fully
    # linear HBM access (strided reads run the DMA engines well below
    # their ~22 GB/s each).
    xt_d = nc.dram_tensor("xt", [XT_TOT], f16, kind="ExternalInput").ap()
    wt_d = nc.dram_tensor("wt", [128, KT * L], f16, kind="ExternalInput").ap()
    dz_d = nc.dram_tensor("dz", [128, 2 * DZW], f16, kind="ExternalInput").ap()
    sp_d = nc.dram_tensor("sp", [OUT_TOT], u8, kind="ExternalOutput").ap()
    gi_d = nc.dram_tensor("gini", [OUT_TOT], u8, kind="ExternalOutput").ap()

    # block start offsets
    starts = []
    n0 = 0
    for nb in BLOCKS:
        starts.append(n0)
        n0 += nb
    assert n0 == NROWS
    CG_END = [CG_OFF[i] + CGS[i] for i in range(len(CGS))]

    def block_of(r):
        for i in range(len(BLOCKS) - 1, -1, -1):
            if r >= starts[i]:
                return i
        raise AssertionError

    with tile.TileContext(nc) as tc:
        with (
            tc.tile_pool(name="consts", bufs=1) as consts,
            tc.tile_pool(name="xt", bufs=4) as xt_pool,
            tc.tile_pool(name="psum", bufs=8, space="PSUM") as psum_pool,
            tc.tile_pool(name="big", bufs=2) as big_pool,
            tc.tile_pool(name="tmp", bufs=2) as tmp_pool,
        ):
            # Persistent last-k-tile buffers: rows 17..127 stay zero so the
            # moving operand always spans 128 partitions; only the 17 real
            # rows are re-DMAed per block (triple-buffered). Memsets first:
            # no deps, and they gate the k6 DMAs of the first blocks.
            xk6s = []
            _ms_engines = [nc.gpsimd, nc.vector, nc.vector]
            for i in range(3):
                t6 = consts.tile([128, 2048], f16, tag=f"xk6_{i}")
                _ms_engines[i].memset(t6[:], 0.0)
                xk6s.append(t6)
            # wt first on the x-load (sync) queue: every matmul needs it;
            # dz later on the scalar queue (first use is ~15us in).
            wt_sb = consts.tile([128, KT, L], f16)
            nc.sync.dma_start(wt_sb[:].rearrange("p k l -> p (k l)"), wt_d[:])
            dz_sb = consts.tile([128, 2, DZW], f16)

            TC = 1024  # tail sub-piece width

            def make_tail(m, off, w, sp_ts):
                """Build the elementwise tail for the cg at rows [off, off+w)
                as a list of piece-closures plus a stores-closure. Pieces are
                interleaved between the NEXT cg's psum-drain ACT ops so the
                in-order ACT queue never holds a tanh batch in front of a
                pending psum drain (which would stall the PE on PSUM)."""
                TCw = min(TC, w)
                gi_ts = [
                    big_pool.tile(
                        [128, w], u8, tag=f"gi{h}", name=f"gi{h}", bufs=3
                    )
                    for h in range(2)
                ]

                def piece(nh, hf):
                    def run():
                        sl = slice(hf * TCw, (hf + 1) * TCw)
                        z = tmp_pool.tile(
                            [128, TCw], f16, tag=f"z{hf}", name="z"
                        )
                        nc.vector.scalar_tensor_tensor(
                            z[:],
                            sp_ts[nh][:, sl],
                            127.5,
                            dz_sb[:, nh, :TCw],
                            Alu.subtract,
                            Alu.mult,
                        )
                        th = tmp_pool.tile(
                            [128, TCw], f16, tag=f"th{hf}", name="th"
                        )
                        nc.scalar.activation(th[:], z[:], Act.Tanh, scale=0.5)
                        nc.vector.scalar_tensor_tensor(
                            gi_ts[nh][:, sl], th[:], 255.0, th[:],
                            Alu.mult, Alu.mult,
                        )

                    return run

                def stores():
                    for nh in range(2):
                        o = OUT_OFF[(m, nh)]
                        dst_sp = sp_d[o : o + 128 * w].rearrange(
                            "(p f) -> p f", p=128
                        )
                        dst_gi = gi_d[o : o + 128 * w].rearrange(
                            "(p f) -> p f", p=128
                        )
                        nc.gpsimd.dma_start(dst_sp, sp_ts[nh][:])
                        nc.gpsimd.dma_start(dst_gi, gi_ts[nh][:])

                return [piece(nh, hf) for nh in range(2)
                        for hf in range(w // TCw)], stores

            xts_by_block = {}
            pending_tail = None
            store_q = []
            emitted_cg = 0
            for bi, nb in enumerate(BLOCKS):
                b0 = starts[bi]
                xts = []
                # Per-DMA descriptor-generation costs ~0.65us of engine time:
                # spread issues across engines (scalar only during the ramp,
                # it is ACT-busy later; sync+gpsimd alternate in steady state).
                if bi < 2:
                    k_eng = [nc.sync, nc.sync, nc.scalar, nc.scalar,
                             nc.gpsimd, nc.gpsimd]
                else:
                    # x split across BOTH hardware DMA queues (sync q1 and
                    # scalar q10): each queue is processing-rate-capped at
                    # ~19-22 GB/s per engine, and the queues run concurrently.
                    k_eng = [nc.sync, nc.scalar, nc.sync, nc.scalar,
                             nc.sync, nc.scalar]
                for k in range(KT - 1):
                    xk = xt_pool.tile([128, nb], f16, tag=f"x{k}", bufs=6)
                    o = XT_OFF[(bi, k)]
                    src = xt_d[o : o + 128 * nb].rearrange("(p f) -> p f", p=128)
                    k_eng[k].dma_start(xk[:], src)
                    xts.append(xk)
                xk6 = xk6s[bi % 3]
                o = XT_OFF[(bi, KT - 1)]
                src6 = xt_d[o : o + KP * nb].rearrange("(p f) -> p f", p=KP)
                nc.sync.dma_start(xk6[0:KP, :nb], src6)
                xts.append(xk6)
                xts_by_block[bi] = xts
                if bi == 1:
                    # dz load off the ramp's critical path
                    nc.scalar.dma_start(
                        dz_sb[:].rearrange("p n f -> p (n f)"), dz_d[:]
                    )

                while emitted_cg < len(CGS) and CG_END[emitted_cg] <= b0 + nb:
                    off, w = CG_OFF[emitted_cg], CGS[emitted_cg]
                    m = emitted_cg
                    while len(store_q) > 1:
                        store_q.pop(0)()
                    sp_ts = []
                    for nh in range(2):
                        sp_t = big_pool.tile(
                            [128, w], u8, tag=f"sp{nh}", bufs=3
                        )
                        sp_ts.append(sp_t)
                        for ch in range(w // CH):
                            r = off + ch * CH
                            cb = block_of(r)
                            lo = r - starts[cb]
                            cxts = xts_by_block[cb]
                            ps = psum_pool.tile([128, CH], f32)
                            for k in range(KT):
                                nc.tensor.matmul(
                                    ps[:],
                                    wt_sb[:, k, nh * 128 : (nh + 1) * 128],
                                    cxts[k][:, lo : lo + CH],
                                    start=(k == 0),
                                    stop=(k == KT - 1),
                                )
                            nc.scalar.activation(
                                sp_t[:, ch * CH : (ch + 1) * CH],
                                ps[:],
                                Act.Copy,
                                bias=127.5,
                                scale=127.5,
                            )
                    if pending_tail is not None:
                        for p in pending_tail[0]:
                            p()
                        store_q.append(pending_tail[1])
                    pending_tail = make_tail(emitted_cg, off, w, sp_ts)
                    emitted_cg += 1
            if pending_tail is not None:
                for p in pending_tail[0]:
                    p()
                store_q.append(pending_tail[1])
            for s in store_q:
                s()

    nc.compile()
    return nc


def _prep_core_x(x_flat_core):
    """[16384, 784] fp32 -> blocked transposed fp16 flat buffer.

    Per block: k-tiles [128, nb] (f on partitions) then the remainder tile
    [17, nb] whose row 16 is the all-ones bias-fold row. Fully contiguous
    per DMA.
    """
    xsT16 = x_flat_core.T.astype(np.float16)  # [784, n], one strided pass
    out = np.empty(XT_TOT, np.float16)
    b0 = 0
    for bi, nb in enumerate(BLOCKS):
        for k in range(6):
            o = XT_OFF[(bi, k)]
            out[o : o + 128 * nb] = xsT16[
                k * 128 : (k + 1) * 128, b0 : b0 + nb
            ].reshape(-1)
        o = XT_OFF[(bi, KT - 1)]
        rem = np.empty((KP, nb), np.float16)
        rem[:16] = xsT16[768:784, b0 : b0 + nb]
        rem[16] = 1.0
        out[o : o + KP * nb] = rem.reshape(-1)
        b0 += nb
    return out


def _prep_wt(W, b):
    wt = np.zeros((KT, 128, L), np.float16)
    WT = W.T  # [784, 256]
    for k in range(6):
        wt[k] = WT[k * 128 : (k + 1) * 128]
    wt[6, :16] = WT[768:784]
    wt[6, 16] = b
    # device layout [128, KT*L]: partition = f-within-tile, free = (k, l)
    return np.ascontiguousarray(wt.transpose(1, 0, 2).reshape(128, KT * L))


_module_cache = {}


def _get_module():
    if "m" not in _module_cache:
        _module_cache["m"] = _build_module()
    return _module_cache["m"]


def _install_ntff_hook():
    """Register the axon NTFF profiling hook missing from this image's antenv."""
    try:
        import antenv.axon_hooks  # noqa: F401

        return
    except ImportError:
        pass
    try:
        from trn_agent_boot.trn_boot import _ntff_profile_via_ctypes

        hook = _ntff_profile_via_ctypes("/opt/axon/libaxon_pjrt.so")
    except Exception:
        hook = None
    mod = types.ModuleType("antenv.axon_hooks")
    mod.get_axon_ntff_profile_hook = lambda: hook
    mod.set_axon_ntff_profile_hook = lambda h: None
    sys.modules["antenv.axon_hooks"] = mod


def _unstage_core(args):
    """Flat blocked u8 pair -> (sp [NROWS,256] f32, gini [NROWS,256] f32)."""
    sp_u8, gi_u8 = args

    def to_lr(flat):
        arr = np.empty((2 * 128, NROWS), np.uint8)
        for m, w in enumerate(CGS):
            off = CG_OFF[m]
            for nh in range(2):
                o = OUT_OFF[(m, nh)]
                arr[nh * 128 : (nh + 1) * 128, off : off + w] = flat[
                    o : o + 128 * w
                ].reshape(128, w)
        return arr

    sp = (to_lr(sp_u8).T.astype(np.float32) - 127.5) * (1.0 / 127.5)
    gi = 1.5 - to_lr(gi_u8).T.astype(np.float32) * (0.5 / 255.0)
    return sp, gi


def _run(x, W, b, contribution, trace=False, tmpdir=None):
    from concourse import bass_utils

    nc = _get_module()

    x_flat = np.ascontiguousarray(x, dtype=np.float32).reshape(NCORES, NROWS, F)
    wt = _prep_wt(np.asarray(W, np.float32), np.asarray(b, np.float32))
    c = np.asarray(contribution, np.float32)
    d = c[:, :, 0] - c[:, :, 1]                      # [T, L]
    dT = (d.T * (1.0 / 127.5)).astype(np.float16)    # [L, T]
    # host layout [128, 2*DZW]: partition p holds [nh=0 reps | nh=1 reps]
    dz = np.ascontiguousarray(
        np.broadcast_to(
            dT.reshape(2, 128, 1, 128).transpose(1, 0, 2, 3),
            (128, 2, DZW // 128, 128),
        ).reshape(128, 2 * DZW)
    )

    with ThreadPoolExecutor(NCORES) as ex:
        xts = list(ex.map(_prep_core_x, [x_flat[i] for i in range(NCORES)]))

    if trace:
        _install_ntff_hook()
    in_maps = [{"xt": xts[i], "wt": wt, "dz": dz} for i in range(NCORES)]
    res = bass_utils.run_bass_kernel_spmd(
        nc, in_maps, core_ids=list(range(NCORES)), trace=trace, tmpdir=tmpdir
    )

    with ThreadPoolExecutor(NCORES) as ex:
        outs = list(
            ex.map(
                _unstage_core,
                [
                    (res.results[i]["sp"], res.results[i]["gini"])
                    for i in range(NCORES)
                ],
            )
        )
    sp = np.concatenate([o[0] for o in outs]).reshape(B, T, L)
    gini = np.concatenate([o[1] for o in outs]).reshape(B, T, L)
    out = (sp, gini)
    return (out, res) if trace else (out, None)


def kernel(x, W, b, contribution):
    out, _ = _run(x, W, b, contribution, trace=False)
    return out


# revision 77
# speedup vs baseline: 1.1125x; 1.1125x over previous
"""Trainium2 Bass kernel for nn_Decision_Node (Linear+Hardtanh -> sp, 2-class
softmax Gini -> gini), data-parallel over 8 NeuronCores.

Math per core shard (B_s=128 of B=1024 batches, T=128, F=784, L=256, C=2):
    sp   = clip(x @ W.T + b, -1, 1)                      [N=16384, 256]
    gini = 1.5 - 0.5*tanh(sp*d/2)^2,  d = contrib[...,0]-contrib[...,1]

Device strategy (v2, W-stationary):
  - x cast to fp16 on host, column-blocked+padded to [7, 128, N] with a
    bias-fold row (xt[6,16,:] = 1.0 pairs with wt[6,16,:] = b).
  - Matmuls keep W tiles stationary ([128f x 128l] from resident SBUF) and
    stream 512-row x chunks; out psum is [128l x 512r] (one full bank).
  - ACT reads each psum chunk once: sp_u8 = sat_u8(round(127.5*y + 127.5)).
    The u8 SATURATION implements the hardtanh clip; round-to-nearest was
    verified on HW. Host dequant: sp = (u8-127.5)/127.5 (max err 3.9e-3).
  - DVE scalar_tensor_tensor: z = (sp_u8 - 127.5) * (d/127.5)  [= sp*d].
  - ACT: th = tanh(z/2).  DVE stt: gini_u8 = (th*255)*th (round+sat).
    Host: gini = 1.5 - 0.5*u8/255.
  - Outputs staged [2(l-half), 128(l), N(rows)] u8; 4 KiB-per-partition
    stores every 4096 rows. Host transposes u8 slabs on dequant.
  - Blocks 1024/2048 rows: small first blocks cut the DMA ramp, small last
    blocks shorten the drain tail.
"""

import os
import sys
import types
from concurrent.futures import ThreadPoolExecutor

import numpy as np

for _p in (
    "/opt/trn_rl_repo",
    "/root/.axon_site",
    "/root/.axon_site/_ro/trn_rl_repo",
    "/root/.axon_site/_ro/pypackages",
):
    if os.path.isdir(_p) and _p not in sys.path:
        sys.path.append(_p)

B, T, F, L = 1024, 128, 784, 256
NCORES = 8
BS = B // NCORES          # batches per core
NROWS = BS * T            # 16384 rows per core
KT = 7                    # contraction tiles (784 = 6*128 + 16, padded)
KP = 17                   # used partitions in the last (remainder+bias) k-tile
CG = 2048                 # compute-group rows (elementwise op width)
CH = 512                  # matmul chunk rows (one psum bank)
DZW = 1024                # dz tile reps (d pattern is 128-periodic in rows)
BLOCKS = [512, 1024, 1536] + [2048] * 6 + [1024]

# flat blocked x layout: per block, k-tiles 0..5 are [128, nb] and the
# remainder k-tile is [KP, nb], all concatenated contiguously so every DMA
# is a fully linear HBM read.
XT_OFF = {}
_o = 0
for _bi, _nb in enumerate(BLOCKS):
    for _k in range(KT - 1):
        XT_OFF[(_bi, _k)] = _o
        _o += 128 * _nb
    XT_OFF[(_bi, KT - 1)] = _o
    _o += KP * _nb
XT_TOT = _o

# compute groups (small final groups shorten the post-matmul drain)
CGS = [CG] * 7 + [1024, 512, 512]
CG_OFF = []
_o = 0
for _w in CGS:
    CG_OFF.append(_o)
    _o += _w
assert _o == NROWS

# flat blocked output layout: per cg, per l-half, a contiguous [128, w] chunk
OUT_OFF = {}
_o = 0
for _m, _w in enumerate(CGS):
    for _nh in range(2):
        OUT_OFF[(_m, _nh)] = _o
        _o += 128 * _w
OUT_TOT = _o
assert OUT_TOT == 2 * 128 * NROWS


def _build_module():
    """Build + compile the single-core Bass/Tile module (SPMD across cores)."""
    import concourse.tile as tile
    from concourse import bacc, mybir

    f32, f16, u8 = mybir.dt.float32, mybir.dt.float16, mybir.dt.uint8
    Alu = mybir.AluOpType
    Act = mybir.ActivationFunctionType

    nc = bacc.Bacc(
        "TRN2",
        target_bir_lowering=False,
        debug=False,
        enable_asserts=False,
        num_devices=NCORES,
    )
    # All dram tensors are laid out by the host so every DMA is a fully
    # linear HBM access (strided reads run the DMA engines well below
    # their ~22 GB/s each).
    xt_d = nc.dram_tensor("xt", [XT_TOT], f16, kind="ExternalInput").ap()
    wt_d = nc.dram_tensor("wt", [128, KT * L], f16, kind="ExternalInput").ap()
    dz_d = nc.dram_tensor("dz", [128, 2 * DZW], f16, kind="ExternalInput").ap()
    sp_d = nc.dram_tensor("sp", [OUT_TOT], u8, kind="ExternalOutput").ap()
    gi_d = nc.dram_tensor("gini", [OUT_TOT], u8, kind="ExternalOutput").ap()

    # block start offsets
    starts = []
    n0 = 0
    for nb in BLOCKS:
        starts.append(n0)
        n0 += nb
    assert n0 == NROWS
    CG_END = [CG_OFF[i] + CGS[i] for i in range(len(CGS))]

    def block_of(r):
        for i in range(len(BLOCKS) - 1, -1, -1):
            if r >= starts[i]:
                return i
        raise AssertionError

    with tile.TileContext(nc) as tc:
        with (
            tc.tile_pool(name="consts", bufs=1) as consts,
            tc.tile_pool(name="xt", bufs=4) as xt_pool,
            tc.tile_pool(name="psum", bufs=8, space="PSUM") as psum_pool,
            tc.tile_pool(name="big", bufs=2) as big_pool,
            tc.tile_pool(name="tmp", bufs=2) as tmp_pool,
        ):
            # Persistent last-k-tile buffers: rows 17..127 stay zero so the
            # moving operand always spans 128 partitions; only the 17 real
            # rows are re-DMAed per block (triple-buffered). Memsets first:
            # no deps, and they gate the k6 DMAs of the first blocks.
            xk6s = []
            _ms_engines = [nc.gpsimd, nc.vector, nc.vector]
            for i in range(3):
                t6 = consts.tile([128, 2048], f16, tag=f"xk6_{i}")
                _ms_engines[i].memset(t6[:], 0.0)
                xk6s.append(t6)
            # wt first on the x-load (sync) queue: every matmul needs it;
            # dz later on the scalar queue (first use is ~15us in).
            wt_sb = consts.tile([128, KT, L], f16)
            nc.sync.dma_start(wt_sb[:].rearrange("p k l -> p (k l)"), wt_d[:])
            dz_sb = consts.tile([128, 2, DZW], f16)

            TC = 1024  # tail sub-piece width

            def make_tail(m, off, w, sp_ts):
                """Build the elementwise tail for the cg at rows [off, off+w)
                as a list of piece-closures plus a stores-closure. Pieces are
                interleaved between the NEXT cg's psum-drain ACT ops so the
                in-order ACT queue never holds a tanh batch in front of a
                pending psum drain (which would stall the PE on PSUM)."""
                TCw = min(TC, w)
                gi_ts = [
                    big_pool.tile(
                        [128, w], u8, tag=f"gi{h}", name=f"gi{h}", bufs=3
                    )
                    for h in range(2)
                ]

                def piece(nh, hf):
                    def run():
                        sl = slice(hf * TCw, (hf + 1) * TCw)
                        z = tmp_pool.tile(
                            [128, TCw], f16, tag=f"z{hf}", name="z"
                        )
                        nc.vector.scalar_tensor_tensor(
                            z[:],
                            sp_ts[nh][:, sl],
                            127.5,
                            dz_sb[:, nh, :TCw],
                            Alu.subtract,
                            Alu.mult,
                        )
                        th = tmp_pool.tile(
                            [128, TCw], f16, tag=f"th{hf}", name="th"
                        )
                        nc.scalar.activation(th[:], z[:], Act.Tanh, scale=0.5)
                        nc.vector.scalar_tensor_tensor(
                            gi_ts[nh][:, sl], th[:], 255.0, th[:],
                            Alu.mult, Alu.mult,
                        )

                    return run

                def stores():
                    for nh in range(2):
                        o = OUT_OFF[(m, nh)]
                        dst_sp = sp_d[o : o + 128 * w].rearrange(
                            "(p f) -> p f", p=128
                        )
                        dst_gi = gi_d[o : o + 128 * w].rearrange(
                            "(p f) -> p f", p=128
                        )
                        nc.gpsimd.dma_start(dst_sp, sp_ts[nh][:])
                        nc.gpsimd.dma_start(dst_gi, gi_ts[nh][:])

                return [piece(nh, hf) for nh in range(2)
                        for hf in range(w // TCw)], stores

            xts_by_block = {}
            pending_tail = None
            store_q = []
            emitted_cg = 0
            for bi, nb in enumerate(BLOCKS):
                b0 = starts[bi]
                xts = []
                # Per-DMA descriptor-generation costs ~0.65us of engine time:
                # spread issues across engines (scalar only during the ramp,
                # it is ACT-busy later; sync+gpsimd alternate in steady state).
                if bi < 2:
                    k_eng = [nc.sync, nc.sync, nc.scalar, nc.scalar,
                             nc.gpsimd, nc.gpsimd]
                else:
                    k_eng = [nc.sync] * 6
                for k in range(KT - 1):
                    xk = xt_pool.tile([128, nb], f16, tag=f"x{k}", bufs=6)
                    o = XT_OFF[(bi, k)]
                    src = xt_d[o : o + 128 * nb].rearrange("(p f) -> p f", p=128)
                    k_eng[k].dma_start(xk[:], src)
                    xts.append(xk)
                xk6 = xk6s[bi % 3]
                o = XT_OFF[(bi, KT - 1)]
                src6 = xt_d[o : o + KP * nb].rearrange("(p f) -> p f", p=KP)
                nc.sync.dma_start(xk6[0:KP, :nb], src6)
                xts.append(xk6)
                xts_by_block[bi] = xts
                if bi == 1:
                    # dz load off the ramp's critical path
                    nc.scalar.dma_start(
                        dz_sb[:].rearrange("p n f -> p (n f)"), dz_d[:]
                    )

                while emitted_cg < len(CGS) and CG_END[emitted_cg] <= b0 + nb:
                    off, w = CG_OFF[emitted_cg], CGS[emitted_cg]
                    m = emitted_cg
                    while len(store_q) > 1:
                        store_q.pop(0)()
                    sp_ts = []
                    for nh in range(2):
                        sp_t = big_pool.tile(
                            [128, w], u8, tag=f"sp{nh}", bufs=3
                        )
                        sp_ts.append(sp_t)
                        for ch in range(w // CH):
                            r = off + ch * CH
                            cb = block_of(r)
                            lo = r - starts[cb]
                            cxts = xts_by_block[cb]
                            ps = psum_pool.tile([128, CH], f32)
                            for k in range(KT):
                                nc.tensor.matmul(
                                    ps[:],
                                    wt_sb[:, k, nh * 128 : (nh + 1) * 128],
                                    cxts[k][:, lo : lo + CH],
                                    start=(k == 0),
                                    stop=(k == KT - 1),
                                )
                            nc.scalar.activation(
                                sp_t[:, ch * CH : (ch + 1) * CH],
                                ps[:],
                                Act.Copy,
                                bias=127.5,
                                scale=127.5,
                            )
                    if pending_tail is not None:
                        for p in pending_tail[0]:
                            p()
                        store_q.append(pending_tail[1])
                    pending_tail = make_tail(emitted_cg, off, w, sp_ts)
                    emitted_cg += 1
            if pending_tail is not None:
                for p in pending_tail[0]:
                    p()
                store_q.append(pending_tail[1])
            for s in store_q:
                s()

    nc.compile()
    return nc


def _prep_core_x(x_flat_core):
    """[16384, 784] fp32 -> blocked transposed fp16 flat buffer.

    Per block: k-tiles [128, nb] (f on partitions) then the remainder tile
    [17, nb] whose row 16 is the all-ones bias-fold row. Fully contiguous
    per DMA.
    """
    xsT16 = x_flat_core.T.astype(np.float16)  # [784, n], one strided pass
    out = np.empty(XT_TOT, np.float16)
    b0 = 0
    for bi, nb in enumerate(BLOCKS):
        for k in range(6):
            o = XT_OFF[(bi, k)]
            out[o : o + 128 * nb] = xsT16[
                k * 128 : (k + 1) * 128, b0 : b0 + nb
            ].reshape(-1)
        o = XT_OFF[(bi, KT - 1)]
        rem = np.empty((KP, nb), np.float16)
        rem[:16] = xsT16[768:784, b0 : b0 + nb]
        rem[16] = 1.0
        out[o : o + KP * nb] = rem.reshape(-1)
        b0 += nb
    return out


def _prep_wt(W, b):
    wt = np.zeros((KT, 128, L), np.float16)
    WT = W.T  # [784, 256]
    for k in range(6):
        wt[k] = WT[k * 128 : (k + 1) * 128]
    wt[6, :16] = WT[768:784]
    wt[6, 16] = b
    # device layout [128, KT*L]: partition = f-within-tile, free = (k, l)
    return np.ascontiguousarray(wt.transpose(1, 0, 2).reshape(128, KT * L))


_module_cache = {}


def _get_module():
    if "m" not in _module_cache:
        _module_cache["m"] = _build_module()
    return _module_cache["m"]


def _install_ntff_hook():
    """Register the axon NTFF profiling hook missing from this image's antenv."""
    try:
        import antenv.axon_hooks  # noqa: F401

        return
    except ImportError:
        pass
    try:
        from trn_agent_boot.trn_boot import _ntff_profile_via_ctypes

        hook = _ntff_profile_via_ctypes("/opt/axon/libaxon_pjrt.so")
    except Exception:
        hook = None
    mod = types.ModuleType("antenv.axon_hooks")
    mod.get_axon_ntff_profile_hook = lambda: hook
    mod.set_axon_ntff_profile_hook = lambda h: None
    sys.modules["antenv.axon_hooks"] = mod


def _unstage_core(args):
    """Flat blocked u8 pair -> (sp [NROWS,256] f32, gini [NROWS,256] f32)."""
    sp_u8, gi_u8 = args

    def to_lr(flat):
        arr = np.empty((2 * 128, NROWS), np.uint8)
        for m, w in enumerate(CGS):
            off = CG_OFF[m]
            for nh in range(2):
                o = OUT_OFF[(m, nh)]
                arr[nh * 128 : (nh + 1) * 128, off : off + w] = flat[
                    o : o + 128 * w
                ].reshape(128, w)
        return arr

    sp = (to_lr(sp_u8).T.astype(np.float32) - 127.5) * (1.0 / 127.5)
    gi = 1.5 - to_lr(gi_u8).T.astype(np.float32) * (0.5 / 255.0)
    return sp, gi


def _run(x, W, b, contribution, trace=False, tmpdir=None):
    from concourse import bass_utils

    nc = _get_module()

    x_flat = np.ascontiguousarray(x, dtype=np.float32).reshape(NCORES, NROWS, F)
    wt = _prep_wt(np.asarray(W, np.float32), np.asarray(b, np.float32))
    c = np.asarray(contribution, np.float32)
    d = c[:, :, 0] - c[:, :, 1]                      # [T, L]
    dT = (d.T * (1.0 / 127.5)).astype(np.float16)    # [L, T]
    # host layout [128, 2*DZW]: partition p holds [nh=0 reps | nh=1 reps]
    dz = np.ascontiguousarray(
        np.broadcast_to(
            dT.reshape(2, 128, 1, 128).transpose(1, 0, 2, 3),
            (128, 2, DZW // 128, 128),
        ).reshape(128, 2 * DZW)
    )

    with ThreadPoolExecutor(NCORES) as ex:
        xts = list(ex.map(_prep_core_x, [x_flat[i] for i in range(NCORES)]))

    if trace:
        _install_ntff_hook()
    in_maps = [{"xt": xts[i], "wt": wt, "dz": dz} for i in range(NCORES)]
    res = bass_utils.run_bass_kernel_spmd(
        nc, in_maps, core_ids=list(range(NCORES)), trace=trace, tmpdir=tmpdir
    )

    with ThreadPoolExecutor(NCORES) as ex:
        outs = list(
            ex.map(
                _unstage_core,
                [
                    (res.results[i]["sp"], res.results[i]["gini"])
                    for i in range(NCORES)
                ],
            )
        )
    sp = np.concatenate([o[0] for o in outs]).reshape(B, T, L)
    gini = np.concatenate([o[1] for o in outs]).reshape(B, T, L)
    out = (sp, gini)
    return (out, res) if trace else (out, None)


def kernel(x, W, b, contribution):
    out, _ = _run(x, W, b, contribution, trace=False)
    return out


# revision 79
# speedup vs baseline: 1.1507x; 1.0344x over previous
"""Trainium2 Bass kernel for nn_Decision_Node (Linear+Hardtanh -> sp, 2-class
softmax Gini -> gini), data-parallel over 8 NeuronCores.

Math per core shard (B_s=128 of B=1024 batches, T=128, F=784, L=256, C=2):
    sp   = clip(x @ W.T + b, -1, 1)                      [N=16384, 256]
    gini = 1.5 - 0.5*tanh(sp*d/2)^2,  d = contrib[...,0]-contrib[...,1]

Device strategy (v2, W-stationary):
  - x cast to fp16 on host, column-blocked+padded to [7, 128, N] with a
    bias-fold row (xt[6,16,:] = 1.0 pairs with wt[6,16,:] = b).
  - Matmuls keep W tiles stationary ([128f x 128l] from resident SBUF) and
    stream 512-row x chunks; out psum is [128l x 512r] (one full bank).
  - ACT reads each psum chunk once: sp_u8 = sat_u8(round(127.5*y + 127.5)).
    The u8 SATURATION implements the hardtanh clip; round-to-nearest was
    verified on HW. Host dequant: sp = (u8-127.5)/127.5 (max err 3.9e-3).
  - DVE scalar_tensor_tensor: z = (sp_u8 - 127.5) * (d/127.5)  [= sp*d].
  - ACT: th = tanh(z/2).  DVE stt: gini_u8 = (th*255)*th (round+sat).
    Host: gini = 1.5 - 0.5*u8/255.
  - Outputs staged [2(l-half), 128(l), N(rows)] u8; 4 KiB-per-partition
    stores every 4096 rows. Host transposes u8 slabs on dequant.
  - Blocks 1024/2048 rows: small first blocks cut the DMA ramp, small last
    blocks shorten the drain tail.
"""

import os
import sys
import types
from concurrent.futures import ThreadPoolExecutor

import numpy as np

for _p in (
    "/opt/trn_rl_repo",
    "/root/.axon_site",
    "/root/.axon_site/_ro/trn_rl_repo",
    "/root/.axon_site/_ro/pypackages",
):
    if os.path.isdir(_p) and _p not in sys.path:
        sys.path.append(_p)

B, T, F, L = 1024, 128, 784, 256
NCORES = 8
BS = B // NCORES          # batches per core
NROWS = BS * T            # 16384 rows per core
KT = 7                    # contraction tiles (784 = 6*128 + 16, padded)
KP = 17                   # used partitions in the last (remainder+bias) k-tile
CG = 2048                 # compute-group rows (elementwise op width)
CH = 512                  # matmul chunk rows (one psum bank)
DZW = 1024                # dz tile reps (d pattern is 128-periodic in rows)
BLOCKS = [512, 1024, 1536] + [2048] * 6 + [1024]

# flat blocked x layout: per block, k-tiles 0..5 are [128, nb] and the
# remainder k-tile is [KP, nb], all concatenated contiguously so every DMA
# is a fully linear HBM read.
XT_OFF = {}
_o = 0
for _bi, _nb in enumerate(BLOCKS):
    for _k in range(KT - 1):
        XT_OFF[(_bi, _k)] = _o
        _o += 128 * _nb
    XT_OFF[(_bi, KT - 1)] = _o
    _o += KP * _nb
XT_TOT = _o

# compute groups (small final groups shorten the post-matmul drain)
CGS = [CG] * 7 + [1024, 512, 512]
CG_OFF = []
_o = 0
for _w in CGS:
    CG_OFF.append(_o)
    _o += _w
assert _o == NROWS

# flat blocked output layout: per cg, per l-half, a contiguous [128, w] chunk
OUT_OFF = {}
_o = 0
for _m, _w in enumerate(CGS):
    for _nh in range(2):
        OUT_OFF[(_m, _nh)] = _o
        _o += 128 * _w
OUT_TOT = _o
assert OUT_TOT == 2 * 128 * NROWS


def _build_module():
    """Build + compile the single-core Bass/Tile module (SPMD across cores)."""
    import concourse.tile as tile
    from concourse import bacc, mybir

    f32, f16, u8 = mybir.dt.float32, mybir.dt.float16, mybir.dt.uint8
    Alu = mybir.AluOpType
    Act = mybir.ActivationFunctionType

    nc = bacc.Bacc(
        "TRN2",
        target_bir_lowering=False,
        debug=False,
        enable_asserts=False,
        num_devices=NCORES,
    )
    # All dram tensors are laid out by the host so every DMA is a fully
    # linear HBM access (strided reads run the DMA engines well below
    # their ~22 GB/s each).
    xt_d = nc.dram_tensor("xt", [XT_TOT], f16, kind="ExternalInput").ap()
    wt_d = nc.dram_tensor("wt", [128, KT * L], f16, kind="ExternalInput").ap()
    dz_d = nc.dram_tensor("dz", [128, 2 * DZW], f16, kind="ExternalInput").ap()
    sp_d = nc.dram_tensor("sp", [OUT_TOT], u8, kind="ExternalOutput").ap()
    gi_d = nc.dram_tensor("gini", [OUT_TOT], u8, kind="ExternalOutput").ap()

    # block start offsets
    starts = []
    n0 = 0
    for nb in BLOCKS:
        starts.append(n0)
        n0 += nb
    assert n0 == NROWS
    CG_END = [CG_OFF[i] + CGS[i] for i in range(len(CGS))]

    def block_of(r):
        for i in range(len(BLOCKS) - 1, -1, -1):
            if r >= starts[i]:
                return i
        raise AssertionError

    with tile.TileContext(nc) as tc:
        with (
            tc.tile_pool(name="consts", bufs=1) as consts,
            tc.tile_pool(name="xt", bufs=4) as xt_pool,
            tc.tile_pool(name="psum", bufs=8, space="PSUM") as psum_pool,
            tc.tile_pool(name="big", bufs=2) as big_pool,
            tc.tile_pool(name="tmp", bufs=2) as tmp_pool,
        ):
            # Persistent last-k-tile buffers: rows 17..127 stay zero so the
            # moving operand always spans 128 partitions; only the 17 real
            # rows are re-DMAed per block (triple-buffered). Memsets first:
            # no deps, and they gate the k6 DMAs of the first blocks.
            xk6s = []
            _ms_engines = [nc.gpsimd, nc.vector, nc.vector]
            for i in range(3):
                t6 = consts.tile([128, 2048], f16, tag=f"xk6_{i}")
                _ms_engines[i].memset(t6[:], 0.0)
                xk6s.append(t6)
            # wt first on the x-load (sync) queue: every matmul needs it;
            # dz later on the scalar queue (first use is ~15us in).
            wt_sb = consts.tile([128, KT, L], f16)
            nc.sync.dma_start(wt_sb[:].rearrange("p k l -> p (k l)"), wt_d[:])
            dz_sb = consts.tile([128, 2, DZW], f16)

            TC = 1024  # tail sub-piece width

            def make_tail(m, off, w, sp_ts):
                """Build the elementwise tail for the cg at rows [off, off+w)
                as a list of piece-closures plus a stores-closure. Pieces are
                interleaved between the NEXT cg's psum-drain ACT ops so the
                in-order ACT queue never holds a tanh batch in front of a
                pending psum drain (which would stall the PE on PSUM)."""
                TCw = min(TC, w)
                gi_ts = [
                    big_pool.tile(
                        [128, w], u8, tag=f"gi{h}", name=f"gi{h}", bufs=3
                    )
                    for h in range(2)
                ]

                def piece(nh, hf):
                    def run():
                        sl = slice(hf * TCw, (hf + 1) * TCw)
                        z = tmp_pool.tile(
                            [128, TCw], f16, tag=f"z{hf}", name="z"
                        )
                        nc.vector.scalar_tensor_tensor(
                            z[:],
                            sp_ts[nh][:, sl],
                            127.5,
                            dz_sb[:, nh, :TCw],
                            Alu.subtract,
                            Alu.mult,
                        )
                        th = tmp_pool.tile(
                            [128, TCw], f16, tag=f"th{hf}", name="th"
                        )
                        nc.scalar.activation(th[:], z[:], Act.Tanh, scale=0.5)
                        nc.vector.scalar_tensor_tensor(
                            gi_ts[nh][:, sl], th[:], 255.0, th[:],
                            Alu.mult, Alu.mult,
                        )

                    return run

                def stores():
                    # stores are latency-tolerant: they share the sync
                    # hardware queue so x alone owns the gpsimd queue
                    for nh in range(2):
                        o = OUT_OFF[(m, nh)]
                        dst_sp = sp_d[o : o + 128 * w].rearrange(
                            "(p f) -> p f", p=128
                        )
                        dst_gi = gi_d[o : o + 128 * w].rearrange(
                            "(p f) -> p f", p=128
                        )
                        nc.sync.dma_start(dst_sp, sp_ts[nh][:])
                        nc.sync.dma_start(dst_gi, gi_ts[nh][:])

                return [piece(nh, hf) for nh in range(2)
                        for hf in range(w // TCw)], stores

            xts_by_block = {}
            pending_tail = None
            store_q = []
            emitted_cg = 0
            for bi, nb in enumerate(BLOCKS):
                b0 = starts[bi]
                xts = []
                # Per-DMA descriptor-generation costs ~0.65us of engine time:
                # spread issues across engines (scalar only during the ramp,
                # it is ACT-busy later; sync+gpsimd alternate in steady state).
                if bi < 2:
                    k_eng = [nc.sync, nc.sync, nc.scalar, nc.scalar,
                             nc.gpsimd, nc.gpsimd]
                else:
                    # Queue balance: the per-engine q1 (hardware, sync) and
                    # q0 (software, gpsimd) DMA queues process concurrently
                    # at ~19 and ~24 GB/s. x k0-k3+k6 ride q0 ALONE (no
                    # head-of-line blocking behind stores); k4,k5 plus all
                    # stores share q1. Balanced, both drain in ~50us instead
                    # of x serializing on q1 for ~108us.
                    k_eng = [nc.gpsimd, nc.gpsimd, nc.gpsimd, nc.gpsimd,
                             nc.sync, nc.sync]
                for k in range(KT - 1):
                    xk = xt_pool.tile([128, nb], f16, tag=f"x{k}", bufs=6)
                    o = XT_OFF[(bi, k)]
                    src = xt_d[o : o + 128 * nb].rearrange("(p f) -> p f", p=128)
                    k_eng[k].dma_start(xk[:], src)
                    xts.append(xk)
                xk6 = xk6s[bi % 3]
                o = XT_OFF[(bi, KT - 1)]
                src6 = xt_d[o : o + KP * nb].rearrange("(p f) -> p f", p=KP)
                (nc.gpsimd if bi >= 2 else nc.sync).dma_start(
                    xk6[0:KP, :nb], src6
                )
                xts.append(xk6)
                xts_by_block[bi] = xts
                if bi == 1:
                    # dz load off the ramp's critical path
                    nc.scalar.dma_start(
                        dz_sb[:].rearrange("p n f -> p (n f)"), dz_d[:]
                    )

                while emitted_cg < len(CGS) and CG_END[emitted_cg] <= b0 + nb:
                    off, w = CG_OFF[emitted_cg], CGS[emitted_cg]
                    m = emitted_cg
                    while len(store_q) > 1:
                        store_q.pop(0)()
                    sp_ts = []
                    for nh in range(2):
                        sp_t = big_pool.tile(
                            [128, w], u8, tag=f"sp{nh}", bufs=3
                        )
                        sp_ts.append(sp_t)
                        for ch in range(w // CH):
                            r = off + ch * CH
                            cb = block_of(r)
                            lo = r - starts[cb]
                            cxts = xts_by_block[cb]
                            ps = psum_pool.tile([128, CH], f32)
                            for k in range(KT):
                                nc.tensor.matmul(
                                    ps[:],
                                    wt_sb[:, k, nh * 128 : (nh + 1) * 128],
                                    cxts[k][:, lo : lo + CH],
                                    start=(k == 0),
                                    stop=(k == KT - 1),
                                )
                            nc.scalar.activation(
                                sp_t[:, ch * CH : (ch + 1) * CH],
                                ps[:],
                                Act.Copy,
                                bias=127.5,
                                scale=127.5,
                            )
                    if pending_tail is not None:
                        for p in pending_tail[0]:
                            p()
                        store_q.append(pending_tail[1])
                    pending_tail = make_tail(emitted_cg, off, w, sp_ts)
                    emitted_cg += 1
            if pending_tail is not None:
                for p in pending_tail[0]:
                    p()
                store_q.append(pending_tail[1])
            for s in store_q:
                s()

    nc.compile()
    return nc


def _prep_core_x(x_flat_core):
    """[16384, 784] fp32 -> blocked transposed fp16 flat buffer.

    Per block: k-tiles [128, nb] (f on partitions) then the remainder tile
    [17, nb] whose row 16 is the all-ones bias-fold row. Fully contiguous
    per DMA.
    """
    xsT16 = x_flat_core.T.astype(np.float16)  # [784, n], one strided pass
    out = np.empty(XT_TOT, np.float16)
    b0 = 0
    for bi, nb in enumerate(BLOCKS):
        for k in range(6):
            o = XT_OFF[(bi, k)]
            out[o : o + 128 * nb] = xsT16[
                k * 128 : (k + 1) * 128, b0 : b0 + nb
            ].reshape(-1)
        o = XT_OFF[(bi, KT - 1)]
        rem = np.empty((KP, nb), np.float16)
        rem[:16] = xsT16[768:784, b0 : b0 + nb]
        rem[16] = 1.0
        out[o : o + KP * nb] = rem.reshape(-1)
        b0 += nb
    return out


def _prep_wt(W, b):
    wt = np.zeros((KT, 128, L), np.float16)
    WT = W.T  # [784, 256]
    for k in range(6):
        wt[k] = WT[k * 128 : (k + 1) * 128]
    wt[6, :16] = WT[768:784]
    wt[6, 16] = b
    # device layout [128, KT*L]: partition = f-within-tile, free = (k, l)
    return np.ascontiguousarray(wt.transpose(1, 0, 2).reshape(128, KT * L))


_module_cache = {}


def _get_module():
    if "m" not in _module_cache:
        _module_cache["m"] = _build_module()
    return _module_cache["m"]


def _install_ntff_hook():
    """Register the axon NTFF profiling hook missing from this image's antenv."""
    try:
        import antenv.axon_hooks  # noqa: F401

        return
    except ImportError:
        pass
    try:
        from trn_agent_boot.trn_boot import _ntff_profile_via_ctypes

        hook = _ntff_profile_via_ctypes("/opt/axon/libaxon_pjrt.so")
    except Exception:
        hook = None
    mod = types.ModuleType("antenv.axon_hooks")
    mod.get_axon_ntff_profile_hook = lambda: hook
    mod.set_axon_ntff_profile_hook = lambda h: None
    sys.modules["antenv.axon_hooks"] = mod


def _unstage_core(args):
    """Flat blocked u8 pair -> (sp [NROWS,256] f32, gini [NROWS,256] f32)."""
    sp_u8, gi_u8 = args

    def to_lr(flat):
        arr = np.empty((2 * 128, NROWS), np.uint8)
        for m, w in enumerate(CGS):
            off = CG_OFF[m]
            for nh in range(2):
                o = OUT_OFF[(m, nh)]
                arr[nh * 128 : (nh + 1) * 128, off : off + w] = flat[
                    o : o + 128 * w
                ].reshape(128, w)
        return arr

    sp = (to_lr(sp_u8).T.astype(np.float32) - 127.5) * (1.0 / 127.5)
    gi = 1.5 - to_lr(gi_u8).T.astype(np.float32) * (0.5 / 255.0)
    return sp, gi


def _run(x, W, b, contribution, trace=False, tmpdir=None):
    from concourse import bass_utils

    nc = _get_module()

    x_flat = np.ascontiguousarray(x, dtype=np.float32).reshape(NCORES, NROWS, F)
    wt = _prep_wt(np.asarray(W, np.float32), np.asarray(b, np.float32))
    c = np.asarray(contribution, np.float32)
    d = c[:, :, 0] - c[:, :, 1]                      # [T, L]
    dT = (d.T * (1.0 / 127.5)).astype(np.float16)    # [L, T]
    # host layout [128, 2*DZW]: partition p holds [nh=0 reps | nh=1 reps]
    dz = np.ascontiguousarray(
        np.broadcast_to(
            dT.reshape(2, 128, 1, 128).transpose(1, 0, 2, 3),
            (128, 2, DZW // 128, 128),
        ).reshape(128, 2 * DZW)
    )

    with ThreadPoolExecutor(NCORES) as ex:
        xts = list(ex.map(_prep_core_x, [x_flat[i] for i in range(NCORES)]))

    if trace:
        _install_ntff_hook()
    in_maps = [{"xt": xts[i], "wt": wt, "dz": dz} for i in range(NCORES)]
    res = bass_utils.run_bass_kernel_spmd(
        nc, in_maps, core_ids=list(range(NCORES)), trace=trace, tmpdir=tmpdir
    )

    with ThreadPoolExecutor(NCORES) as ex:
        outs = list(
            ex.map(
                _unstage_core,
                [
                    (res.results[i]["sp"], res.results[i]["gini"])
                    for i in range(NCORES)
                ],
            )
        )
    sp = np.concatenate([o[0] for o in outs]).reshape(B, T, L)
    gini = np.concatenate([o[1] for o in outs]).reshape(B, T, L)
    out = (sp, gini)
    return (out, res) if trace else (out, None)


def kernel(x, W, b, contribution):
    out, _ = _run(x, W, b, contribution, trace=False)
    return out


# revision 84
# speedup vs baseline: 1.1702x; 1.0169x over previous
"""Trainium2 Bass kernel for nn_Decision_Node (Linear+Hardtanh -> sp, 2-class
softmax Gini -> gini), data-parallel over 8 NeuronCores.

Math per core shard (B_s=128 of B=1024 batches, T=128, F=784, L=256, C=2):
    sp   = clip(x @ W.T + b, -1, 1)                      [N=16384, 256]
    gini = 1.5 - 0.5*tanh(sp*d/2)^2,  d = contrib[...,0]-contrib[...,1]

Device strategy (v2, W-stationary):
  - x cast to fp16 on host, column-blocked+padded to [7, 128, N] with a
    bias-fold row (xt[6,16,:] = 1.0 pairs with wt[6,16,:] = b).
  - Matmuls keep W tiles stationary ([128f x 128l] from resident SBUF) and
    stream 512-row x chunks; out psum is [128l x 512r] (one full bank).
  - ACT reads each psum chunk once: sp_u8 = sat_u8(round(127.5*y + 127.5)).
    The u8 SATURATION implements the hardtanh clip; round-to-nearest was
    verified on HW. Host dequant: sp = (u8-127.5)/127.5 (max err 3.9e-3).
  - DVE scalar_tensor_tensor: z = (sp_u8 - 127.5) * (d/127.5)  [= sp*d].
  - ACT: th = tanh(z/2).  DVE stt: gini_u8 = (th*255)*th (round+sat).
    Host: gini = 1.5 - 0.5*u8/255.
  - Outputs staged [2(l-half), 128(l), N(rows)] u8; 4 KiB-per-partition
    stores every 4096 rows. Host transposes u8 slabs on dequant.
  - Blocks 1024/2048 rows: small first blocks cut the DMA ramp, small last
    blocks shorten the drain tail.
"""

import os
import sys
import types
from concurrent.futures import ThreadPoolExecutor

import numpy as np

for _p in (
    "/opt/trn_rl_repo",
    "/root/.axon_site",
    "/root/.axon_site/_ro/trn_rl_repo",
    "/root/.axon_site/_ro/pypackages",
):
    if os.path.isdir(_p) and _p not in sys.path:
        sys.path.append(_p)

B, T, F, L = 1024, 128, 784, 256
NCORES = 8
BS = B // NCORES          # batches per core
NROWS = BS * T            # 16384 rows per core
KT = 7                    # contraction tiles (784 = 6*128 + 16, padded)
KP = 17                   # used partitions in the last (remainder+bias) k-tile
CG = 2048                 # compute-group rows (elementwise op width)
CH = 512                  # matmul chunk rows (one psum bank)
DZW = 1024                # dz tile reps (d pattern is 128-periodic in rows)
BLOCKS = [512, 1024, 1536] + [2048] * 6 + [1024]

# flat blocked x layout: per block, k-tiles 0..5 are [128, nb] and the
# remainder k-tile is [KP, nb], all concatenated contiguously so every DMA
# is a fully linear HBM read.
XT_OFF = {}
_o = 0
for _bi, _nb in enumerate(BLOCKS):
    for _k in range(KT - 1):
        XT_OFF[(_bi, _k)] = _o
        _o += 128 * _nb
    XT_OFF[(_bi, KT - 1)] = _o
    _o += KP * _nb
XT_TOT = _o

# compute groups (small final groups shorten the post-matmul drain)
CGS = [CG] * 7 + [1024, 512, 512]
CG_OFF = []
_o = 0
for _w in CGS:
    CG_OFF.append(_o)
    _o += _w
assert _o == NROWS

# flat blocked output layout: per cg, per l-half, a contiguous [128, w] chunk
OUT_OFF = {}
_o = 0
for _m, _w in enumerate(CGS):
    for _nh in range(2):
        OUT_OFF[(_m, _nh)] = _o
        _o += 128 * _w
OUT_TOT = _o
assert OUT_TOT == 2 * 128 * NROWS


def _build_module():
    """Build + compile the single-core Bass/Tile module (SPMD across cores)."""
    import concourse.tile as tile
    from concourse import bacc, mybir

    f32, f16, u8 = mybir.dt.float32, mybir.dt.float16, mybir.dt.uint8
    Alu = mybir.AluOpType
    Act = mybir.ActivationFunctionType

    nc = bacc.Bacc(
        "TRN2",
        target_bir_lowering=False,
        debug=False,
        enable_asserts=False,
        num_devices=NCORES,
    )
    # All dram tensors are laid out by the host so every DMA is a fully
    # linear HBM access (strided reads run the DMA engines well below
    # their ~22 GB/s each).
    xt_d = nc.dram_tensor("xt", [XT_TOT], f16, kind="ExternalInput").ap()
    wt_d = nc.dram_tensor("wt", [128, KT * L], f16, kind="ExternalInput").ap()
    dz_d = nc.dram_tensor("dz", [128, 2 * DZW], f16, kind="ExternalInput").ap()
    sp_d = nc.dram_tensor("sp", [OUT_TOT], u8, kind="ExternalOutput").ap()
    gi_d = nc.dram_tensor("gini", [OUT_TOT], u8, kind="ExternalOutput").ap()

    # block start offsets
    starts = []
    n0 = 0
    for nb in BLOCKS:
        starts.append(n0)
        n0 += nb
    assert n0 == NROWS
    CG_END = [CG_OFF[i] + CGS[i] for i in range(len(CGS))]

    def block_of(r):
        for i in range(len(BLOCKS) - 1, -1, -1):
            if r >= starts[i]:
                return i
        raise AssertionError

    with tile.TileContext(nc) as tc:
        with (
            tc.tile_pool(name="consts", bufs=1) as consts,
            tc.tile_pool(name="xt", bufs=4) as xt_pool,
            tc.tile_pool(name="psum", bufs=8, space="PSUM") as psum_pool,
            tc.tile_pool(name="big", bufs=2) as big_pool,
            tc.tile_pool(name="tmp", bufs=2) as tmp_pool,
        ):
            # Persistent last-k-tile buffers: rows 17..127 stay zero so the
            # moving operand always spans 128 partitions; only the 17 real
            # rows are re-DMAed per block (triple-buffered). Memsets first:
            # no deps, and they gate the k6 DMAs of the first blocks.
            xk6s = []
            _ms_engines = [nc.gpsimd, nc.vector, nc.vector]
            for i in range(3):
                t6 = consts.tile([128, 2048], f16, tag=f"xk6_{i}")
                _ms_engines[i].memset(t6[:], 0.0)
                xk6s.append(t6)
            # wt first on the x-load (sync) queue: every matmul needs it;
            # dz later on the scalar queue (first use is ~15us in).
            wt_sb = consts.tile([128, KT, L], f16)
            nc.sync.dma_start(wt_sb[:].rearrange("p k l -> p (k l)"), wt_d[:])
            dz_sb = consts.tile([128, 2, DZW], f16)

            TC = 1024  # tail sub-piece width

            def make_tail(m, off, w, sp_ts):
                """Build the elementwise tail for the cg at rows [off, off+w)
                as a list of piece-closures plus a stores-closure. Pieces are
                interleaved between the NEXT cg's psum-drain ACT ops so the
                in-order ACT queue never holds a tanh batch in front of a
                pending psum drain (which would stall the PE on PSUM)."""
                TCw = min(TC, w)
                gi_ts = [
                    big_pool.tile(
                        [128, w], u8, tag=f"gi{h}", name=f"gi{h}", bufs=3
                    )
                    for h in range(2)
                ]

                def piece(nh, hf):
                    def run():
                        sl = slice(hf * TCw, (hf + 1) * TCw)
                        z = tmp_pool.tile(
                            [128, TCw], f16, tag=f"z{hf}", name="z"
                        )
                        nc.vector.scalar_tensor_tensor(
                            z[:],
                            sp_ts[nh][:, sl],
                            127.5,
                            dz_sb[:, nh, :TCw],
                            Alu.subtract,
                            Alu.mult,
                        )
                        th = tmp_pool.tile(
                            [128, TCw], f16, tag=f"th{hf}", name="th"
                        )
                        nc.scalar.activation(th[:], z[:], Act.Tanh, scale=0.5)
                        nc.vector.scalar_tensor_tensor(
                            gi_ts[nh][:, sl], th[:], 255.0, th[:],
                            Alu.mult, Alu.mult,
                        )

                    return run

                def stores():
                    # stores are latency-tolerant: they share the sync
                    # hardware queue so x alone owns the gpsimd queue
                    for nh in range(2):
                        o = OUT_OFF[(m, nh)]
                        dst_sp = sp_d[o : o + 128 * w].rearrange(
                            "(p f) -> p f", p=128
                        )
                        dst_gi = gi_d[o : o + 128 * w].rearrange(
                            "(p f) -> p f", p=128
                        )
                        nc.sync.dma_start(dst_sp, sp_ts[nh][:])
                        nc.sync.dma_start(dst_gi, gi_ts[nh][:])

                return [piece(nh, hf) for nh in range(2)
                        for hf in range(w // TCw)], stores

            xts_by_block = {}
            pending_tail = None
            store_q = []
            emitted_cg = 0
            for bi, nb in enumerate(BLOCKS):
                b0 = starts[bi]
                xts = []
                # Per-DMA descriptor-generation costs ~0.65us of engine time:
                # spread issues across engines (scalar only during the ramp,
                # it is ACT-busy later; sync+gpsimd alternate in steady state).
                if bi < 2:
                    k_eng = [nc.sync, nc.sync, nc.scalar, nc.scalar,
                             nc.gpsimd, nc.gpsimd]
                else:
                    # Queue balance: the per-engine q1 (hardware, sync) and
                    # q0 (software, gpsimd) DMA queues process concurrently
                    # at ~19 and ~24 GB/s. x k0-k3+k6 ride q0 ALONE (no
                    # head-of-line blocking behind stores); k4,k5 plus all
                    # stores share q1. Balanced, both drain in ~50us instead
                    # of x serializing on q1 for ~108us.
                    k_eng = [nc.gpsimd, nc.gpsimd, nc.gpsimd, nc.gpsimd,
                             nc.sync, nc.sync]
                for k in range(KT - 1):
                    xk = xt_pool.tile([128, nb], f16, tag=f"x{k}", bufs=6)
                    o = XT_OFF[(bi, k)]
                    src = xt_d[o : o + 128 * nb].rearrange("(p f) -> p f", p=128)
                    k_eng[k].dma_start(xk[:], src)
                    xts.append(xk)
                xk6 = xk6s[bi % 3]
                o = XT_OFF[(bi, KT - 1)]
                src6 = xt_d[o : o + KP * nb].rearrange("(p f) -> p f", p=KP)
                (nc.gpsimd if bi >= 2 else nc.sync).dma_start(
                    xk6[0:KP, :nb], src6
                )
                xts.append(xk6)
                xts_by_block[bi] = xts
                if bi == 1:
                    # dz load off the ramp's critical path
                    nc.scalar.dma_start(
                        dz_sb[:].rearrange("p n f -> p (n f)"), dz_d[:]
                    )

                while emitted_cg < len(CGS) and CG_END[emitted_cg] <= b0 + nb:
                    off, w = CG_OFF[emitted_cg], CGS[emitted_cg]
                    m = emitted_cg
                    while len(store_q) > 1:
                        store_q.pop(0)()
                    sp_ts = []
                    for nh in range(2):
                        sp_t = big_pool.tile(
                            [128, w], u8, tag=f"sp{nh}", bufs=3
                        )
                        sp_ts.append(sp_t)
                        for ch in range(w // CH):
                            r = off + ch * CH
                            cb = block_of(r)
                            lo = r - starts[cb]
                            cxts = xts_by_block[cb]
                            ps = psum_pool.tile([128, CH], f32)
                            for k in range(KT):
                                nc.tensor.matmul(
                                    ps[:],
                                    wt_sb[:, k, nh * 128 : (nh + 1) * 128],
                                    cxts[k][:, lo : lo + CH],
                                    start=(k == 0),
                                    stop=(k == KT - 1),
                                )
                            nc.scalar.activation(
                                sp_t[:, ch * CH : (ch + 1) * CH],
                                ps[:],
                                Act.Copy,
                                bias=127.5,
                                scale=127.5,
                            )
                    if pending_tail is not None:
                        for p in pending_tail[0]:
                            p()
                        store_q.append(pending_tail[1])
                    pending_tail = make_tail(emitted_cg, off, w, sp_ts)
                    emitted_cg += 1
            if pending_tail is not None:
                for p in pending_tail[0]:
                    p()
                store_q.append(pending_tail[1])
            for s in store_q:
                s()

    nc.compile()
    return nc


def _prep_core_x(x_flat_core):
    """[16384, 784] fp32 -> blocked transposed fp16 flat buffer.

    Per block: k-tiles [128, nb] (f on partitions) then the remainder tile
    [17, nb] whose row 16 is the all-ones bias-fold row. Fully contiguous
    per DMA.
    """
    xsT16 = x_flat_core.T.astype(np.float16)  # [784, n], one strided pass
    out = np.empty(XT_TOT, np.float16)
    b0 = 0
    for bi, nb in enumerate(BLOCKS):
        for k in range(6):
            o = XT_OFF[(bi, k)]
            out[o : o + 128 * nb] = xsT16[
                k * 128 : (k + 1) * 128, b0 : b0 + nb
            ].reshape(-1)
        o = XT_OFF[(bi, KT - 1)]
        rem = np.empty((KP, nb), np.float16)
        rem[:16] = xsT16[768:784, b0 : b0 + nb]
        rem[16] = 1.0
        out[o : o + KP * nb] = rem.reshape(-1)
        b0 += nb
    return out


def _prep_wt(W, b):
    wt = np.zeros((KT, 128, L), np.float16)
    WT = W.T  # [784, 256]
    for k in range(6):
        wt[k] = WT[k * 128 : (k + 1) * 128]
    wt[6, :16] = WT[768:784]
    wt[6, 16] = b
    # device layout [128, KT*L]: partition = f-within-tile, free = (k, l)
    return np.ascontiguousarray(wt.transpose(1, 0, 2).reshape(128, KT * L))


_module_cache = {}


def _get_module():
    if "m" not in _module_cache:
        _module_cache["m"] = _build_module()
    return _module_cache["m"]


def _install_ntff_hook():
    """Register the axon NTFF profiling hook missing from this image's antenv."""
    try:
        import antenv.axon_hooks  # noqa: F401

        return
    except ImportError:
        pass
    try:
        from trn_agent_boot.trn_boot import _ntff_profile_via_ctypes

        hook = _ntff_profile_via_ctypes("/opt/axon/libaxon_pjrt.so")
    except Exception:
        hook = None
    mod = types.ModuleType("antenv.axon_hooks")
    mod.get_axon_ntff_profile_hook = lambda: hook
    mod.set_axon_ntff_profile_hook = lambda h: None
    sys.modules["antenv.axon_hooks"] = mod


def _unstage_core(args):
    """Flat blocked u8 pair -> (sp [NROWS,256] f32, gini [NROWS,256] f32)."""
    sp_u8, gi_u8 = args

    def to_lr(flat):
        arr = np.empty((2 * 128, NROWS), np.uint8)
        for m, w in enumerate(CGS):
            off = CG_OFF[m]
            for nh in range(2):
                o = OUT_OFF[(m, nh)]
                arr[nh * 128 : (nh + 1) * 128, off : off + w] = flat[
                    o : o + 128 * w
                ].reshape(128, w)
        return arr

    sp = (to_lr(sp_u8).T.astype(np.float32) - 127.5) * (1.0 / 127.5)
    gi = 1.5 - to_lr(gi_u8).T.astype(np.float32) * (0.5 / 255.0)
    return sp, gi


def _run(x, W, b, contribution, trace=False, tmpdir=None):
    from concourse import bass_utils

    nc = _get_module()

    x_flat = np.ascontiguousarray(x, dtype=np.float32).reshape(NCORES, NROWS, F)
    wt = _prep_wt(np.asarray(W, np.float32), np.asarray(b, np.float32))
    c = np.asarray(contribution, np.float32)
    d = c[:, :, 0] - c[:, :, 1]                      # [T, L]
    dT = (d.T * (1.0 / 127.5)).astype(np.float16)    # [L, T]
    # host layout [128, 2*DZW]: partition p holds [nh=0 reps | nh=1 reps]
    dz = np.ascontiguousarray(
        np.broadcast_to(
            dT.reshape(2, 128, 1, 128).transpose(1, 0, 2, 3),
            (128, 2, DZW // 128, 128),
        ).reshape(128, 2 * DZW)
    )

    with ThreadPoolExecutor(NCORES) as ex:
        xts = list(ex.map(_prep_core_x, [x_flat[i] for i in range(NCORES)]))

    if trace:
        _install_ntff_hook()
    in_maps = [{"xt": xts[i], "wt": wt, "dz": dz} for i in range(NCORES)]
    res = bass_utils.run_bass_kernel_spmd(
        nc, in_maps, core_ids=list(range(NCORES)), trace=trace, tmpdir=tmpdir
    )

    with ThreadPoolExecutor(NCORES) as ex:
        outs = list(
            ex.map(
                _unstage_core,
                [
                    (res.results[i]["sp"], res.results[i]["gini"])
                    for i in range(NCORES)
                ],
            )
        )
    sp = np.concatenate([o[0] for o in outs]).reshape(B, T, L)
    gini = np.concatenate([o[1] for o in outs]).reshape(B, T, L)
    out = (sp, gini)
    return (out, res) if trace else (out, None)


def kernel(x, W, b, contribution):
    out, _ = _run(x, W, b, contribution, trace=False)
    return out
